# revision 10
# baseline (speedup 1.0000x reference)
"""Causal multi-head attention (B=2, T=2048, C=1024, H=16) on 8 trn2 cores.

Sharding: core d handles batch b = d//4 and heads hs = [4*(d%4) .. 4*(d%4)+3].
Each core computes the QKV projection for its heads, causal flash-style
attention, and a partial output projection; the host sums the 4 partials
per batch and adds the bias row.

Device-side layouts (per core):
  xT    [C, T]      x[b] transposed (host-prepped)
  qkT   [512, T]    K then Q, per-head transposed [hd, T] rows
  v     [T, 4*65]   V natural layout per head + ones column (softmax denom)
  scoresT[k, q] = K_h @ Q_h^T computed per 128-row k-block, exp'd on ACT,
  causal diag masked by affine_select; EV matmul accumulates out^T [65, q]
  in PSUM with row 64 = softmax denominator; divide-normalize on DVE;
  output projection contracts head-pair channel tiles (K=128).
"""

import sys

sys.path.insert(0, "/opt/trn_rl_repo")

from contextlib import ExitStack

import numpy as np

import concourse.bacc as bacc
import concourse.mybir as mybir
import concourse.tile as tile
from concourse import bass_utils
from concourse._compat import axon_active

B, T, C, H, HD = 2, 2048, 1024, 16, 64
N_CORES = 8
HPD = 4  # heads per device
LCH = HPD * HD  # local channels = 256

# compute dtype: "f32", "f32r", or "bf16"
COMPUTE = "f32"

dt = mybir.dt
if COMPUTE == "bf16":
    CT = dt.bfloat16
    CT_NP = np.dtype("bfloat16") if hasattr(np, "bfloat16") else None
else:
    CT = dt.float32
    CT_NP = np.float32
MM = dt.float32r if COMPUTE == "f32r" else CT  # matmul operand dtype


def _mm(ap):
    """View an AP in the matmul dtype (bitcast for f32r)."""
    if MM is dt.float32r:
        return ap.bitcast(dt.float32r)
    return ap


def _split512(cs, ce):
    """Split [cs, ce) into <=512 chunks, end-aligned to 512 boundaries."""
    subs = []
    s = cs
    while s < ce:
        e = min(ce, (s // 512 + 1) * 512)
        subs.append((s, e))
        s = e
    return subs


def build_kernel():
    nc = bacc.Bacc(
        "TRN2",
        target_bir_lowering=False,
        debug=False,
        enable_asserts=False,
        num_devices=N_CORES,
    )

    xT_d = nc.dram_tensor("xT", [C, T], CT, kind="ExternalInput").ap()
    wqkT_d = nc.dram_tensor("wqkT", [C, 512], CT, kind="ExternalInput").ap()
    wvT_d = nc.dram_tensor("wvT", [C, LCH], CT, kind="ExternalInput").ap()
    bq_d = nc.dram_tensor("bq", [128, 2], dt.float32, kind="ExternalInput").ap()
    wo_d = nc.dram_tensor("wo", [2, 128, C], CT, kind="ExternalInput").ap()
    y_d = nc.dram_tensor("y", [T, C], dt.float32, kind="ExternalOutput").ap()

    KB = C // 128  # 8 contraction blocks
    TB = T // 128  # 16 T blocks

    with tile.TileContext(nc) as tc, ExitStack() as ctx:
        persist = ctx.enter_context(tc.tile_pool(name="persist", bufs=1))
        scpool = ctx.enter_context(tc.tile_pool(name="scp", bufs=2, space="PSUM"))
        evpool = ctx.enter_context(tc.tile_pool(name="evp", bufs=2, space="PSUM"))
        small = ctx.enter_context(tc.tile_pool(name="small", bufs=2))

        # ---- persistent SBUF tiles ----
        qk_sb = [persist.tile([128, T], CT, tag=f"qk{m}", name=f"qk{m}") for m in range(4)]
        v_sb = [persist.tile([128, HPD * 65], CT, tag=f"v{t}", name=f"v{t}") for t in range(TB)]
        wo_sb = [persist.tile([128, C], CT, tag=f"wo{p}", name=f"wo{p}") for p in range(2)]
        ol_sb = [persist.tile([128, T], CT, tag=f"ol{p}", name=f"ol{p}") for p in range(2)]
        bq_sb = persist.tile([128, 2], dt.float32, tag="bq", name="bq_sb")

        nc.sync.dma_start(bq_sb[:], bq_d[:])
        for p in range(2):
            nc.sync.dma_start(wo_sb[p][:], wo_d[p])
        for t in range(TB):
            # ones columns for the softmax-denominator trick (col 64 of each head slot)
            nc.gpsimd.memset(v_sb[t][:, 64 : HPD * 65 : 65], 1.0)

        # ================= projection phase =================
        with tc.tile_pool(name="proj", bufs=1) as projpool:
            wqk_sb = [projpool.tile([128, 512], CT, tag=f"wqk{k}", name=f"wqks{k}") for k in range(KB)]
            wv_sb = [projpool.tile([128, LCH], CT, tag=f"wv{k}", name=f"wvs{k}") for k in range(KB)]
            for k in range(KB):
                nc.sync.dma_start(wqk_sb[k][:], wqkT_d[128 * k : 128 * (k + 1), :])
                nc.sync.dma_start(wv_sb[k][:], wvT_d[128 * k : 128 * (k + 1), :])

            for qc in range(4):
                xs = [
                    projpool.tile([128, 512], CT, tag=f"xT{k}", bufs=2, name=f"xTs{k}_{qc}")
                    for k in range(KB)
                ]
                for k in range(KB):
                    nc.sync.dma_start(
                        xs[k][:], xT_d[128 * k : 128 * (k + 1), 512 * qc : 512 * (qc + 1)]
                    )

                # qkT = WqkT.T @ xT   -> [512 rows = K(4 heads)|Q(4 heads), T]
                for m in range(4):
                    ps = scpool.tile([128, 1024], dt.float32, tag="sc", name=f"psqk{m}_{qc}")
                    for kb in range(KB):
                        nc.tensor.matmul(
                            ps[:, 0:512],
                            _mm(wqk_sb[kb][:, 128 * m : 128 * (m + 1)]),
                            _mm(xs[kb][:, :]),
                            start=(kb == 0),
                            stop=(kb == KB - 1),
                        )
                    dst = qk_sb[m][:, 512 * qc : 512 * (qc + 1)]
                    if m < 2:  # K rows: plain copy (K bias is softmax-invariant)
                        nc.scalar.copy(dst, ps[:, 0:512])
                    else:  # Q rows: add (pre-scaled) bias
                        nc.vector.tensor_scalar_add(dst, ps[:, 0:512], bq_sb[:, m - 2 : m - 1])

                # V natural = xT.T @ WvT  -> [T, 256] for the 4 T-blocks of this qc
                for tl in range(4):
                    t = 4 * qc + tl
                    ps = scpool.tile([128, 1024], dt.float32, tag="sc", name=f"psv{t}")
                    for kb in range(KB):
                        nc.tensor.matmul(
                            ps[:, 0:LCH],
                            _mm(xs[kb][:, 128 * tl : 128 * (tl + 1)]),
                            _mm(wv_sb[kb][:, :]),
                            start=(kb == 0),
                            stop=(kb == KB - 1),
                        )
                    dst = v_sb[t][:, 0 : HPD * 65].rearrange("p (h c) -> p h c", c=65)[:, :, 0:64]
                    src = ps[:, 0:LCH].rearrange("p (h c) -> p h c", c=64)
                    nc.vector.tensor_copy(dst, src)

        # ================= attention phase =================
        with tc.tile_pool(name="attn", bufs=6) as etpool:
            for h in range(HPD):
                ktile = qk_sb[h // 2]
                qtile = qk_sb[2 + h // 2]
                kb0 = 64 * (h % 2)
                ev = [
                    evpool.tile([65, 1024], dt.float32, tag="ev", name=f"ev0_h{h}"),
                    evpool.tile([65, 1024], dt.float32, tag="ev", name=f"ev1_h{h}"),
                ]
                for k in range(TB):
                    qs = 128 * k
                    eTk = etpool.tile([128, T], CT, tag="eT", name=f"eT_h{h}_k{k}")
                    chunks = [(qs, 1024), (1024, 2048)] if qs < 1024 else [(qs, 2048)]
                    for ci, (cs, ce) in enumerate(chunks):
                        scp = scpool.tile([128, 1024], dt.float32, tag="sc")
                        ca = (cs // 1024) * 1024  # psum-bank-aligned base
                        subs = _split512(cs, ce)
                        for ss, se in subs:
                            nc.tensor.matmul(
                                scp[:, ss - ca : se - ca],
                                _mm(ktile[kb0 : kb0 + 64, 128 * k : 128 * (k + 1)]),
                                _mm(qtile[kb0 : kb0 + 64, ss:se]),
                                start=True,
                                stop=True,
                            )
                        nc.scalar.activation(
                            eTk[:, cs:ce],
                            scp[:, cs - ca : ce - ca],
                            mybir.ActivationFunctionType.Exp,
                        )
                        if ci == 0:
                            # causal diagonal: keep q' >= k' within the [qs, qs+128) block
                            nc.gpsimd.affine_select(
                                eTk[:, qs : qs + 128],
                                eTk[:, qs : qs + 128],
                                compare_op=mybir.AluOpType.is_ge,
                                fill=0.0,
                                base=0,
                                channel_multiplier=-1,
                                pattern=[[1, 128]],
                            )
                        for ss, se in subs:
                            half = 0 if ss < 1024 else 1
                            last_k = 7 if half == 0 else TB - 1
                            nc.tensor.matmul(
                                ev[half][:, ss - 1024 * half : se - 1024 * half],
                                _mm(v_sb[k][:, 65 * h : 65 * h + 65]),
                                _mm(eTk[:, ss:se]),
                                start=(k == 0),
                                stop=(k == last_k and se == ce),
                                skip_group_check=True,
                            )
                    if k == 7:
                        _normalize(nc, small, ev[0], ol_sb, h, 0)
                _normalize(nc, small, ev[1], ol_sb, h, 1)

        # ================= output projection =================
        with tc.tile_pool(name="fin", bufs=4) as finpool:
            for t in range(TB):
                for oc in range(2):
                    ps = scpool.tile([128, 1024], dt.float32, tag="sc")
                    for p in range(2):
                        nc.tensor.matmul(
                            ps[:, 0:512],
                            _mm(ol_sb[p][:, 128 * t : 128 * (t + 1)]),
                            _mm(wo_sb[p][:, 512 * oc : 512 * (oc + 1)]),
                            start=(p == 0),
                            stop=(p == 1),
                        )
                    ysb = finpool.tile([128, 512], dt.float32, tag="y")
                    nc.any.tensor_copy(ysb[:], ps[:, 0:512])
                    nc.sync.dma_start(
                        y_d[128 * t : 128 * (t + 1), 512 * oc : 512 * (oc + 1)], ysb[:]
                    )

    nc.compile()
    return nc


def _normalize(nc, small, evp, ol_sb, h, half):
    """out_localT[h] = ev[0:64] * (1 / ev[64]) (broadcast along partitions)."""
    sums = small.tile([64, 1024], mybir.dt.float32, tag="sum")
    nc.scalar.copy(sums[0:1, :], evp[64:65, :])
    nc.vector.reciprocal(sums[0:1, :], sums[0:1, :])
    bc = small.tile([64, 1024], mybir.dt.float32, tag="bc")
    nc.gpsimd.partition_broadcast(bc[:], sums[:])
    if h % 2 == 0:
        dst = ol_sb[h // 2][0:64, 1024 * half : 1024 * (half + 1)]
        nc.vector.tensor_mul(dst, evp[0:64, :], bc[:])
    else:
        tmp = small.tile([64, 1024], CT, tag="olt_tmp")
        nc.vector.tensor_mul(tmp[:], evp[0:64, :], bc[:])
        nc.sync.dma_start(
            ol_sb[h // 2][64:128, 1024 * half : 1024 * (half + 1)], tmp[:]
        )


_NC_CACHE = None


def _get_nc():
    global _NC_CACHE
    if _NC_CACHE is None:
        _NC_CACHE = build_kernel()
    return _NC_CACHE


def _to_ct(a):
    if COMPUTE == "bf16":
        import ml_dtypes

        return np.ascontiguousarray(a).astype(ml_dtypes.bfloat16)
    return np.ascontiguousarray(a.astype(np.float32))


def make_in_maps(x, W_qkv, b_qkv, W_o):
    x = np.asarray(x, dtype=np.float32)
    W_qkv = np.asarray(W_qkv, dtype=np.float32)
    b_qkv = np.asarray(b_qkv, dtype=np.float32)
    W_o = np.asarray(W_o, dtype=np.float32)

    in_maps = []
    for d in range(N_CORES):
        b = d // 4
        hs = [4 * (d % 4) + i for i in range(HPD)]
        xT = x[b].T  # [C, T]

        k_rows = np.concatenate([W_qkv[64 * h : 64 * h + 64] for h in hs])  # [256, C]
        q_rows = np.concatenate([W_qkv[C + 64 * h : C + 64 * h + 64] for h in hs]) * 0.125
        v_rows = np.concatenate([W_qkv[2 * C + 64 * h : 2 * C + 64 * h + 64] for h in hs])
        wqkT = np.concatenate([k_rows, q_rows]).T  # [C, 512]
        wvT = v_rows.T  # [C, 256]

        bq = np.concatenate([b_qkv[C + 64 * h : C + 64 * h + 64] for h in hs]) * 0.125
        bq_packed = bq.reshape(2, 128).T.copy().astype(np.float32)  # [128, 2]

        # head-pair channel packing for the output projection
        wo = np.stack(
            [
                np.concatenate(
                    [W_o[:, 64 * hs[2 * p + j] : 64 * hs[2 * p + j] + 64].T for j in range(2)]
                )
                for p in range(2)
            ]
        )  # [2, 128, C]

        in_maps.append(
            {
                "xT": _to_ct(xT),
                "wqkT": _to_ct(wqkT),
                "wvT": _to_ct(wvT),
                "bq": bq_packed,
                "wo": _to_ct(wo),
            }
        )
    return in_maps


def assemble(results, b_qkv, W_o, b_o):
    b_qkv = np.asarray(b_qkv, dtype=np.float32)
    W_o = np.asarray(W_o, dtype=np.float32)
    b_o = np.asarray(b_o, dtype=np.float32)
    bv = b_qkv[2 * C :]  # [C]
    bias_row = b_o + bv @ W_o.T  # [C]
    y = np.zeros((B, T, C), dtype=np.float32)
    for d in range(N_CORES):
        y[d // 4] += results[d]["y"]
    y += bias_row[None, None, :]
    return y


def kernel(x, W_qkv, b_qkv, W_o, b_o, _trace=False, _trace_kwargs=None):
    nc = _get_nc()
    in_maps = make_in_maps(x, W_qkv, b_qkv, W_o)
    kwargs = {}
    if _trace:
        kwargs.update(trace=True, **(_trace_kwargs or {}))
    res = bass_utils.run_bass_kernel_spmd(
        nc, in_maps, core_ids=list(range(N_CORES)), **kwargs
    )
    y = assemble(res.results, b_qkv, W_o, b_o)
    if _trace:
        return y, res
    return y
